# revision 1
# baseline (speedup 1.0000x reference)
"""Trainium2 Bass kernel for a transformer decoder block (self-attn + cross-attn + MLP).

Sharding: 8 cores = 2 batch groups x 4 cores. Within a group, core c owns
rows r = c (mod 4) of its batch (strided rows balance causal attention work
while keeping the compiled program identical across cores). K/V are computed
replicated within a group (no collectives; cores are fully independent).

Layouts: activations row-major [rows, feat]; weights host-pre-transposed to
[in_feat, out_feat]; logits computed transposed [keys, rows] so exp(logits)
feeds the A@V matmul directly; the softmax denominator comes from an
appended ones-column in V (row 64 of the A@V output).
"""

import os
import sys

for _p in ("/opt/trn_rl_repo", "/root/.axon_site/_ro/trn_rl_repo"):
    if os.path.isdir(_p) and _p not in sys.path:
        sys.path.insert(0, _p)

import numpy as np

B, N, C, H, Y_DIM, HID = 2, 2048, 1024, 16, 1024, 4096
HD = C // H
SCALE = HD ** -0.5
EPS = 1e-5

G = 2          # batch groups
CPG = 4        # cores per group
R = N // CPG   # rows per core (512)
RT = R // 128  # row tiles per core (4)
KB = N // 128  # key blocks (16)
KIN = C // 128  # contraction tiles for C (8)
NEG = -1e9

_CACHE = {}


# ---------------------------------------------------------------------------
# program builder
# ---------------------------------------------------------------------------

def _build(mode, skip_gb):
    """mode: 'causal' | 'none' | 'dense'"""
    import concourse.bass as bass
    import concourse.mybir as mybir
    import concourse.tile as tile
    from concourse import bacc
    from concourse.masks import make_identity

    dt = mybir.dt
    F32, F32R, BF16 = dt.float32, dt.float32r, dt.bfloat16
    AF = mybir.ActivationFunctionType
    ALU = mybir.AluOpType

    nc = bacc.Bacc("TRN2", target_bir_lowering=False, debug=False, num_devices=8)

    # ---- DRAM I/O ----------------------------------------------------------
    def din(name, shape, dtype=None):
        return nc.dram_tensor(name, list(shape), dtype or F32,
                              kind="ExternalInput").ap()

    x_my = din("x_my", (R, C))
    x_full = din("x_full", (N, C))
    yT = din("yT", (Y_DIM, N))
    wqkT = din("wqkT", (C, 2 * C))
    wvT = din("wvT", (C, C))
    wprojT = din("wprojT", (C, C))
    projb = din("projb", (C,))
    wq2T = din("wq2T", (C, C))
    wkv2T = din("wkv2T", (Y_DIM, 2 * C))
    wproj2T = din("wproj2T", (C, C))
    proj2b = din("proj2b", (C,))
    wfc1T = din("wfc1T", (C, HID))
    fc1b = din("fc1b", (HID,))
    wfc2T = din("wfc2T", (HID, C))
    fc2b = din("fc2b", (C,))
    if mode == "causal":
        bmask = din("bmask", (128, 32))
    if mode == "dense":
        maskT = din("maskT", (N, R))
    if not skip_gb:
        lng = {k: din("g_" + k, (HID if k == "mln2" else C,))
               for k in ("ln1", "aln2", "a2ln", "mln1", "mln2")}
        lnb = {k: din("b_" + k, (HID if k == "mln2" else C,))
               for k in ("ln1", "aln2", "a2ln", "mln1", "mln2")}
    out_my = nc.dram_tensor("out_my", [R, C], F32, kind="ExternalOutput").ap()
    h4_stage = nc.dram_tensor("h4_stage", [R, HID], F32).ap()
    hT_stage = nc.dram_tensor("hT_stage", [C, N], F32).ap()

    def bcast(vec_ap, n):
        # DRAM [n] -> AP replicated across 128 partitions
        return bass.AP(tensor=vec_ap.tensor, offset=vec_ap.offset,
                       ap=[[0, 128]] + vec_ap.ap)

    with tile.TileContext(nc) as tc:
        with tc.tile_pool(name="singles", bufs=1) as singles, \
             tc.tile_pool(name="stats", bufs=4) as stats, \
             tc.tile_pool(name="resid", bufs=1) as resid:

            ident = singles.tile([128, 128], F32, name="ident", tag="ident")
            make_identity(nc, ident)
            eps_t = singles.tile([128, 1], F32, name="eps", tag="eps")
            nc.vector.memset(eps_t, EPS)

            if mode == "causal":
                bmask_t = singles.tile([128, 32], F32, name="bmask", tag="bmask")
                nc.sync.dma_start(out=bmask_t, in_=bmask)
            maskT_t = None
            if mode == "dense":
                maskT_t = [singles.tile([128, R], F32, name=f"maskT{j}", tag=f"maskT{j}")
                           for j in range(KB)]
                for j in range(KB):
                    nc.sync.dma_start(out=maskT_t[j], in_=maskT[j * 128:(j + 1) * 128, :])

            gb_tiles = {}
            if not skip_gb:
                for k in ("ln1", "aln2", "a2ln", "mln1", "mln2"):
                    d = HID if k == "mln2" else C
                    gt = singles.tile([128, d], F32, name=f"g_{k}", tag=f"g_{k}")
                    bt = singles.tile([128, d], F32, name=f"b_{k}", tag=f"b_{k}")
                    nc.sync.dma_start(out=gt, in_=bcast(lng[k], d))
                    nc.sync.dma_start(out=bt, in_=bcast(lnb[k], d))
                    gb_tiles[k] = (gt, bt)

            # ---- helpers ---------------------------------------------------
            def ln_apply(h_out, x_in, d, key):
                """LayerNorm of x_in [128, d] -> h_out. Stats on DVE, apply on ACT."""
                nsub = max(1, d // 512)
                st = stats.tile([128, nsub, 6], F32, name="bnst", tag="bnst")
                if nsub > 1:
                    xr = x_in.rearrange("p (s q) -> p s q", s=nsub)
                    for s in range(nsub):
                        nc.vector.bn_stats(out=st[:, s, :], in_=xr[:, s, :])
                else:
                    nc.vector.bn_stats(out=st[:, 0, :], in_=x_in)
                mv = stats.tile([128, 2], F32, name="bnmv", tag="bnmv")
                nc.vector.bn_aggr(out=mv, in_=st)
                sd = stats.tile([128, 1], F32, name="bnsd", tag="bnsd")
                nc.scalar.activation(out=sd, in_=mv[:, 1:2], func=AF.Sqrt, bias=eps_t)
                ri = stats.tile([128, 1], F32, name="bnri", tag="bnri")
                nc.vector.reciprocal(out=ri, in_=sd)
                nm = stats.tile([128, 1], F32, name="bnnm", tag="bnnm")
                nc.vector.tensor_scalar(out=nm, in0=mv[:, 0:1], scalar1=ri,
                                        scalar2=-1.0, op0=ALU.mult, op1=ALU.mult)
                nc.scalar.activation(out=h_out, in_=x_in, func=AF.Identity,
                                     bias=nm, scale=ri)
                if not skip_gb:
                    gt, bt = gb_tiles[key]
                    nc.vector.tensor_tensor(out=h_out, in0=h_out, in1=gt[:, :d],
                                            op=ALU.mult)
                    nc.vector.tensor_tensor(out=h_out, in0=h_out, in1=bt[:, :d],
                                            op=ALU.add)

            def transpose_128(pp, dst, src_tile, rt_idx, nblk, eng=None):
                """src [128, nblk*128] f32 -> dst [128, nblk, R] at col rt_idx*128,
                written as f32r (dst feeds f32r matmuls). Merged copies."""
                for g in range(0, nblk, 8):
                    nb = min(8, nblk - g)
                    ptb = pp.tile([128, 8, 128], F32, name="tpb", tag="tpb")
                    for k in range(nb):
                        nc.tensor.transpose(
                            ptb[:, k, :],
                            src_tile[:, (g + k) * 128:(g + k + 1) * 128], ident)
                    (eng or nc.vector).tensor_copy(
                        out=dst[:, g:g + nb,
                                rt_idx * 128:(rt_idx + 1) * 128].bitcast(F32R),
                        in_=ptb[:, 0:nb, :])

            def build_qT(pp, wp, qT_t, w_ap, w_col0, hT_tiles):
                """qT_t: KIN tiles [128, R] bf16, pre-scaled by SCALE."""
                for mh in range(2):
                    wts = []
                    for K in range(KIN):
                        wt = wp.tile([128, 512], F32R, name=f"kww{K}", tag=f"kww{K}",
                                     bufs=1)
                        nc.sync.dma_start(
                            out=wt, in_=w_ap[K * 128:(K + 1) * 128,
                                             w_col0 + mh * 512:w_col0 + (mh + 1) * 512].bitcast(F32R))
                        wts.append(wt)
                    for mm in range(4):
                        m = mh * 4 + mm
                        ps = pp.tile([128, R], F32, name="qps", tag="qps")
                        for K in range(KIN):
                            nc.tensor.matmul(ps, wts[K][:, mm * 128:(mm + 1) * 128],
                                             hT_tiles[:, K, :].bitcast(F32R),
                                             start=(K == 0), stop=(K == KIN - 1))
                        nc.scalar.mul(out=qT_t[m], in_=ps, mul=SCALE)

            def build_kT(pp, wp, kT_t, w_ap, w_col0, rhs_loader, kdim):
                """kT_t: KIN dst tiles [128, N] bf16 = (W.T).T @ act.T.
                rhs_loader(n) -> list of kdim//128 chunk APs [128, 512]."""
                nkt = kdim // 128
                for n in range(N // 512):
                    chunks = rhs_loader(n)
                    for mh in range(2):
                        wts = []
                        for K in range(nkt):
                            wt = wp.tile([128, 512], F32R, name=f"kww{K}",
                                         tag=f"kww{K}", bufs=2)
                            nc.sync.dma_start(
                                out=wt, in_=w_ap[K * 128:(K + 1) * 128,
                                                 w_col0 + mh * 512:w_col0 + (mh + 1) * 512].bitcast(F32R))
                            wts.append(wt)
                        for mm in range(4):
                            m = mh * 4 + mm
                            ps = pp.tile([128, 512], F32, name="kps", tag="kps")
                            for K in range(nkt):
                                nc.tensor.matmul(ps, wts[K][:, mm * 128:(mm + 1) * 128],
                                                 chunks[K].bitcast(F32R),
                                                 start=(K == 0), stop=(K == nkt - 1))
                            nc.scalar.copy(out=kT_t[m][:, n * 512:(n + 1) * 512],
                                           in_=ps)

            def build_v(pp, wp, v_t, w_ap, w_col0, lhs_loader, kdim):
                """v_t: N//128 dst tiles [128, H*65] bf16 (row-major V, ones col at 64).
                lhs_loader(t) -> AP [128, nkt, 128] (transposed act blocks)."""
                nkt = kdim // 128
                for half in range(2):
                    wts = []
                    for K in range(nkt):
                        wt = wp.tile([128, 512], F32R, name=f"vw{K}", tag=f"vw{K}", bufs=2)
                        nc.sync.dma_start(
                            out=wt, in_=w_ap[K * 128:(K + 1) * 128,
                                             w_col0 + half * 512:w_col0 + (half + 1) * 512].bitcast(F32R))
                        wts.append(wt)
                    for t in range(N // 128):
                        lhs = lhs_loader(t)
                        ps = pp.tile([128, 512], F32, name="vps", tag="vps")
                        for K in range(nkt):
                            nc.tensor.matmul(ps, lhs[:, K, :],
                                             wts[K].bitcast(F32R),
                                             start=(K == 0), stop=(K == nkt - 1))
                        dst = v_t[t].rearrange("p (h c) -> p h c", c=65)
                        nc.vector.tensor_copy(
                            out=dst[:, half * 8:(half + 1) * 8, 0:64],
                            in_=ps.rearrange("p (h c) -> p h c", c=64))
                for t in range(N // 128):
                    dst = v_t[t].rearrange("p (h c) -> p h c", c=65)
                    nc.vector.memset(dst[:, :, 64:65], 1.0)

            def rows_matmul(pp, wp, lhsT, w_ap, dout, kdim, consume):
                """out[rows, dout] = act @ W.T, row-major psum per (rt, nch).
                lhsT: packed [128, kdim//128, R]. consume(rt, nch, psum)."""
                nkt = kdim // 128
                for nch in range(dout // 512):
                    pss = [pp.tile([128, 512], F32, name=f"dps{rt}", tag=f"dps{rt}") for rt in range(RT)]
                    for K in range(nkt):
                        wt = wp.tile([128, 512], F32R, name="dw", tag="dw")
                        nc.sync.dma_start(
                            out=wt, in_=w_ap[K * 128:(K + 1) * 128,
                                             nch * 512:(nch + 1) * 512].bitcast(F32R))
                        for rt in range(RT):
                            nc.tensor.matmul(
                                pss[rt],
                                lhsT[:, K, rt * 128:(rt + 1) * 128].bitcast(F32R),
                                wt.bitcast(F32R),
                                start=(K == 0), stop=(K == nkt - 1))
                    for rt in range(RT):
                        consume(rt, nch, pss[rt])

            def attention(pool, qT_t, kT_t, v_t, o_sb, causal):
                """o_sb: RT tiles [128, C] f32 <- softmax(qk + mask) @ v, per head."""
                with tc.tile_pool(name="attp", bufs=2, space="PSUM") as pp, \
                     tc.tile_pool(name="attps", bufs=1, space="PSUM") as pp1, \
                     tc.tile_pool(name="attw", bufs=3) as aw:
                    for hp in range(H // 2):
                        o_ps = [pp1.tile([65, R], F32, name=f"ops{hh}", tag=f"ops{hh}") for hh in range(2)]
                        for J in range(KB):
                            r0 = 32 * J if causal else 0
                            nj = R - r0
                            lg = pp.tile([128, 2, 512], F32, name="logits", tag="logits")
                            for hh in range(2):
                                h = 2 * hp + hh
                                lhsT = kT_t[h // 2][(h % 2) * 64:(h % 2) * 64 + 64,
                                                   J * 128:(J + 1) * 128]
                                rhs = qT_t[h // 2][(h % 2) * 64:(h % 2) * 64 + 64, r0:R]
                                nc.tensor.matmul(lg[:, hh, 0:nj], lhsT, rhs)
                            if causal:
                                bm = bass.AP(tensor=bmask_t.tensor, offset=bmask_t.offset,
                                             ap=[bmask_t.ap[0], [0, 2], bmask_t.ap[1]])
                                nc.vector.tensor_tensor(out=lg[:, :, 0:32],
                                                        in0=lg[:, :, 0:32],
                                                        in1=bm, op=ALU.add)
                            if mode == "dense":
                                mt = maskT_t[J]
                                mk = bass.AP(tensor=mt.tensor, offset=mt.offset,
                                             ap=[mt.ap[0], [0, 2], mt.ap[1]])
                                nc.vector.tensor_tensor(out=lg[:, :, 0:nj],
                                                        in0=lg[:, :, 0:nj],
                                                        in1=mk, op=ALU.add)
                            pt = aw.tile([128, 2, 512], BF16, name="probs", tag="probs")
                            nc.scalar.activation(out=pt[:, :, 0:nj], in_=lg[:, :, 0:nj],
                                                 func=AF.Exp)
                            for hh in range(2):
                                h = 2 * hp + hh
                                nc.tensor.matmul(o_ps[hh][:, r0:R],
                                                 v_t[J][:, h * 65:h * 65 + 65],
                                                 pt[:, hh, 0:nj],
                                                 start=(J == 0), stop=(J == KB - 1))
                        for hh in range(2):
                            h = 2 * hp + hh
                            ot = aw.tile([65, R], F32, name="otsb", tag="otsb")
                            nc.vector.tensor_copy(out=ot, in_=o_ps[hh])
                            for t in range(RT):
                                tp = pp.tile([128, 65], F32, name="otp", tag="otp")
                                nc.tensor.transpose(
                                    tp, ot[:, t * 128:(t + 1) * 128],
                                    ident[0:65, 0:65])
                                ri = stats.tile([128, 1], F32, name="osum", tag="osum")
                                nc.vector.reciprocal(out=ri, in_=tp[:, 64:65])
                                nc.vector.tensor_scalar(
                                    out=o_sb[t][:, h * 64:(h + 1) * 64],
                                    in0=tp[:, 0:64], scalar1=ri,
                                    scalar2=None, op0=ALU.mult)

            # ================================================================
            # Stage A: self-attention
            # ================================================================
            x_my_t = [resid.tile([128, C], F32, name=f"xmy{t}", tag=f"xmy{t}") for t in range(RT)]
            x1_my = [resid.tile([128, C], F32, name=f"x1my{t}", tag=f"x1my{t}") for t in range(RT)]
            # x2 reuses x_my's slots (x_my is dead once the self-attn proj consumed it)
            x2_my = [resid.tile([128, C], F32, name=f"x2my{t}", tag=f"xmy{t}") for t in range(RT)]

            with tc.tile_pool(name="akv", bufs=1) as akv:
                qT = [akv.tile([128, R], BF16, name=f"qT{m}", tag=f"qT{m}") for m in range(KIN)]
                kT = [akv.tile([128, N], BF16, name=f"kT{m}", tag=f"kT{m}") for m in range(KIN)]
                v_t = [akv.tile([128, H * 65], BF16, name=f"v{t}", tag=f"v{t}")
                       for t in range(N // 128)]
                o_sb = [akv.tile([128, C], F32, name=f"osb{t}", tag=f"osb{t}") for t in range(RT)]

                if True:
                    with tc.tile_pool(name="aq", bufs=1) as ab, \
                         tc.tile_pool(name="awork", bufs=2) as awk, \
                         tc.tile_pool(name="apsA", bufs=2, space="PSUM") as aps:
                        hmyT = ab.tile([128, KIN, R], F32, name="hmyT", tag="hmyT")
                        for t in range(RT):
                            nc.sync.dma_start(out=x_my_t[t],
                                              in_=x_my[t * 128:(t + 1) * 128, :])
                            hm = awk.tile([128, C], F32, name="hmy", tag="hmy")
                            ln_apply(hm, x_my_t[t], C, "ln1")
                            transpose_128(aps, hmyT, hm, t, KIN)
                        # h for all rows -> transposed -> DRAM staging
                        for t in range(N // 128):
                            xf = awk.tile([128, C], F32, name="xfull", tag="xfull")
                            nc.sync.dma_start(out=xf, in_=x_full[t * 128:(t + 1) * 128, :])
                            hf = awk.tile([128, C], F32, name="hfull", tag="hfull")
                            ln_apply(hf, xf, C, "ln1")
                            hs = awk.tile([128, KIN, 128], F32, name="hstg", tag="hstg")
                            ptb = aps.tile([128, KIN, 128], F32, name="tpb", tag="tpb")
                            for k in range(KIN):
                                nc.tensor.transpose(ptb[:, k, :],
                                                    hf[:, k * 128:(k + 1) * 128], ident)
                            if t % 2 == 0:
                                nc.vector.tensor_copy(out=hs, in_=ptb)
                            else:
                                nc.scalar.copy(out=hs, in_=ptb)
                            nc.sync.dma_start(
                                out=hT_stage.rearrange("(kb p) n -> p kb n", p=128)[
                                    :, :, t * 128:(t + 1) * 128], in_=hs)

                        with tc.tile_pool(name="awtq", bufs=1) as awtq:
                            build_qT(aps, awtq, qT, wqkT, 0, hmyT)

                    with tc.tile_pool(name="awtk", bufs=1) as awtk, \
                         tc.tile_pool(name="apsK", bufs=3, space="PSUM") as aps, \
                         tc.tile_pool(name="ahc", bufs=2) as ahc:
                        def h_chunks(n):
                            out = []
                            for K in range(KIN):
                                hc = ahc.tile([128, 512], F32R, name=f"hc{K}",
                                              tag=f"hc{K}")
                                nc.sync.dma_start(
                                    out=hc, in_=hT_stage[K * 128:(K + 1) * 128,
                                                         n * 512:(n + 1) * 512].bitcast(F32R))
                                out.append(hc)
                            return out

                        build_kT(aps, awtk, kT, wqkT, C, h_chunks, C)

                    with tc.tile_pool(name="awtv", bufs=1) as awtv, \
                         tc.tile_pool(name="avw", bufs=2) as avw, \
                         tc.tile_pool(name="apsV", bufs=3, space="PSUM") as aps:
                        def h_block(t):
                            hb = avw.tile([128, KIN, 128], F32R, name="hb", tag="hb")
                            nc.sync.dma_start(
                                out=hb, in_=hT_stage.rearrange(
                                    "(kb p) n -> p kb n", p=128)[
                                    :, :, t * 128:(t + 1) * 128].bitcast(F32R))
                            return hb

                        build_v(aps, awtv, v_t, wvT, 0, h_block, C)

                attention(None, qT, kT, v_t, o_sb, causal=(mode == "causal"))

                # ln2(o) -> transpose -> proj -> +bias +x_my -> x1_my
                with tc.tile_pool(name="aproj", bufs=1) as apj, \
                     tc.tile_pool(name="apwork", bufs=2) as apw, \
                     tc.tile_pool(name="apwt", bufs=4) as apwt, \
                     tc.tile_pool(name="appsum", bufs=2, space="PSUM") as app, \
                     tc.tile_pool(name="appsum1", bufs=1, space="PSUM") as app1:
                    pb_t = apj.tile([128, C], F32, name="projb", tag="projb")
                    nc.sync.dma_start(out=pb_t, in_=bcast(projb, C))
                    olnT = apj.tile([128, KIN, R], F32, name="olnT", tag="olnT")
                    for t in range(RT):
                        oln = apw.tile([128, C], F32, name="oln", tag="oln")
                        ln_apply(oln, o_sb[t], C, "aln2")
                        transpose_128(app, olnT, oln, t, KIN)

                    def consume_proj(rt, nch, ps):
                        sl = slice(nch * 512, (nch + 1) * 512)
                        nc.vector.tensor_tensor(out=x1_my[rt][:, sl], in0=ps,
                                                in1=pb_t[:, sl], op=ALU.add)
                        nc.vector.tensor_tensor(out=x1_my[rt][:, sl],
                                                in0=x1_my[rt][:, sl],
                                                in1=x_my_t[rt][:, sl], op=ALU.add)

                    rows_matmul(app1, apwt, olnT, wprojT, C, C, consume_proj)

            # ================================================================
            # Stage B: cross-attention
            # ================================================================
            with tc.tile_pool(name="bkv", bufs=1) as bkv:
                q2T = [bkv.tile([128, R], BF16, name=f"q2T{m}", tag=f"q2T{m}") for m in range(KIN)]
                k2T = [bkv.tile([128, N], BF16, name=f"k2T{m}", tag=f"k2T{m}") for m in range(KIN)]
                v2_t = [bkv.tile([128, H * 65], BF16, name=f"v2{t}", tag=f"v2{t}")
                        for t in range(N // 128)]
                o2_sb = [bkv.tile([128, C], F32, name=f"o2sb{t}", tag=f"o2sb{t}") for t in range(RT)]

                if True:
                    with tc.tile_pool(name="bq", bufs=1) as bb, \
                         tc.tile_pool(name="bwork", bufs=2) as bwk, \
                         tc.tile_pool(name="bpsQ", bufs=2, space="PSUM") as bps:
                        h2T = bb.tile([128, KIN, R], F32, name="h2T", tag="h2T")
                        for t in range(RT):
                            h2 = bwk.tile([128, C], F32, name="h2", tag="h2")
                            ln_apply(h2, x1_my[t], C, "a2ln")
                            transpose_128(bps, h2T, h2, t, KIN)
                        with tc.tile_pool(name="bwtq", bufs=1) as bwtq:
                            build_qT(bps, bwtq, q2T, wq2T, 0, h2T)

                    with tc.tile_pool(name="bwtk", bufs=1) as bwtk, \
                         tc.tile_pool(name="bpsK", bufs=3, space="PSUM") as bps, \
                         tc.tile_pool(name="byc", bufs=2) as byc:
                        def y_chunks(n):
                            out = []
                            for K in range(Y_DIM // 128):
                                yc = byc.tile([128, 512], F32R, name=f"yc{K}",
                                              tag=f"yc{K}")
                                nc.sync.dma_start(
                                    out=yc, in_=yT[K * 128:(K + 1) * 128,
                                                   n * 512:(n + 1) * 512].bitcast(F32R))
                                out.append(yc)
                            return out

                        build_kT(bps, bwtk, k2T, wkv2T, 0, y_chunks, Y_DIM)

                    with tc.tile_pool(name="bwtv", bufs=1) as bwtv, \
                         tc.tile_pool(name="bvw", bufs=2) as bvw, \
                         tc.tile_pool(name="bpsV", bufs=3, space="PSUM") as bps:
                        def y_block(t):
                            yb = bvw.tile([128, Y_DIM // 128, 128], F32R, name="yb",
                                          tag="yb")
                            nc.sync.dma_start(
                                out=yb, in_=yT.rearrange("(kb p) n -> p kb n", p=128)[
                                    :, :, t * 128:(t + 1) * 128].bitcast(F32R))
                            return yb

                        build_v(bps, bwtv, v2_t, wkv2T, C, y_block, Y_DIM)

                attention(None, q2T, k2T, v2_t, o2_sb, causal=False)

                with tc.tile_pool(name="bproj", bufs=1) as bpj, \
                     tc.tile_pool(name="bpwork", bufs=2) as bpw, \
                     tc.tile_pool(name="bpwt", bufs=4) as bpwt, \
                     tc.tile_pool(name="bppsum", bufs=2, space="PSUM") as bpp, \
                     tc.tile_pool(name="bppsum1", bufs=1, space="PSUM") as bpp1:
                    p2b_t = bpj.tile([128, C], F32, name="proj2b", tag="proj2b")
                    nc.sync.dma_start(out=p2b_t, in_=bcast(proj2b, C))
                    o2T = bpj.tile([128, KIN, R], F32, name="o2T", tag="o2T")
                    for t in range(RT):
                        transpose_128(bpp, o2T, o2_sb[t], t, KIN)

                    def consume_proj2(rt, nch, ps):
                        sl = slice(nch * 512, (nch + 1) * 512)
                        nc.vector.tensor_tensor(out=x2_my[rt][:, sl], in0=ps,
                                                in1=p2b_t[:, sl], op=ALU.add)
                        nc.vector.tensor_tensor(out=x2_my[rt][:, sl],
                                                in0=x2_my[rt][:, sl],
                                                in1=x1_my[rt][:, sl], op=ALU.add)

                    rows_matmul(bpp1, bpwt, o2T, wproj2T, C, C, consume_proj2)

            # ================================================================
            # Stage C: MLP (h4 staged via DRAM to bound SBUF)
            # ================================================================
            with tc.tile_pool(name="cpool", bufs=1) as cp, \
                 tc.tile_pool(name="cwork", bufs=2) as cw, \
                 tc.tile_pool(name="cwt", bufs=4) as cwt, \
                 tc.tile_pool(name="cpsum", bufs=2, space="PSUM") as cps, \
                 tc.tile_pool(name="cpsum1", bufs=1, space="PSUM") as cps1:
                h3T = cp.tile([128, KIN, R], F32, name="h3T", tag="h3T")
                for t in range(RT):
                    h3 = cw.tile([128, C], F32, name="h3", tag="h3")
                    ln_apply(h3, x2_my[t], C, "mln1")
                    transpose_128(cps, h3T, h3, t, KIN)

                fb1_t = cp.tile([128, HID], F32, name="fc1b", tag="fc1b")
                nc.sync.dma_start(out=fb1_t, in_=bcast(fc1b, HID))

                def consume_fc1(rt, nch, ps):
                    sl = slice(nch * 512, (nch + 1) * 512)
                    g4 = cw.tile([128, 512], F32, name="g4", tag="g4")
                    nc.vector.tensor_tensor(out=ps, in0=ps, in1=fb1_t[:, sl], op=ALU.add)
                    nc.scalar.activation(out=g4, in_=ps, func=AF.Gelu)
                    nc.sync.dma_start(out=h4_stage[rt * 128:(rt + 1) * 128, sl], in_=g4)

                rows_matmul(cps1, cwt, h3T, wfc1T, HID, C, consume_fc1)

                h5T = cp.tile([128, HID // 128, R], F32, name="h5T", tag="h5T")
                for t in range(RT):
                    h4t = cw.tile([128, HID], F32, name="h4t", tag="h4t", bufs=1)
                    nc.sync.dma_start(out=h4t, in_=h4_stage[t * 128:(t + 1) * 128, :])
                    h5 = cw.tile([128, HID], F32, name="h5", tag="h5", bufs=1)
                    ln_apply(h5, h4t, HID, "mln2")
                    transpose_128(cps, h5T, h5, t, HID // 128)

                fb2_t = cp.tile([128, C], F32, name="fc2b", tag="fc2b")
                nc.sync.dma_start(out=fb2_t, in_=bcast(fc2b, C))

                def consume_fc2(rt, nch, ps):
                    sl = slice(nch * 512, (nch + 1) * 512)
                    x3 = cw.tile([128, 512], F32, name="x3", tag="x3")
                    nc.vector.tensor_tensor(out=x3, in0=ps, in1=fb2_t[:, sl], op=ALU.add)
                    nc.vector.tensor_tensor(out=x3, in0=x3, in1=x2_my[rt][:, sl],
                                            op=ALU.add)
                    nc.sync.dma_start(out=out_my[rt * 128:(rt + 1) * 128, sl], in_=x3)

                rows_matmul(cps1, cwt, h5T, wfc2T, C, HID, consume_fc2)

    nc.compile()
    return nc


# ---------------------------------------------------------------------------
# host side
# ---------------------------------------------------------------------------

def _host_prep(inputs):
    f32 = np.float32
    x = np.asarray(inputs["x"], f32)
    y = np.asarray(inputs["y"], f32)
    mask = np.asarray(inputs["mask"])[0, 0]  # [N, N] bool

    causal_ref = np.triu(np.ones((N, N), bool), k=1)
    if np.array_equal(mask, causal_ref):
        mode = "causal"
    elif not mask.any():
        mode = "none"
    else:
        mode = "dense"

    gbs = [("a1_ln1_g", "a1_ln1_b"), ("a1_ln2_g", "a1_ln2_b"),
           ("a2_ln_g", "a2_ln_b"), ("m_ln1_g", "m_ln1_b"), ("m_ln2_g", "m_ln2_b")]
    skip_gb = all(
        np.all(np.asarray(inputs[g]) == 1.0) and np.all(np.asarray(inputs[b]) == 0.0)
        for g, b in gbs)

    wT = lambda k: np.ascontiguousarray(np.asarray(inputs[k], f32).T)
    shared = {
        "wqkT": wT("a1_qk_w"),      # [C, 2C]: cols 0:C = q, C:2C = k
        "wvT": wT("a1_v_w"),
        "wprojT": wT("a1_proj_w"),
        "projb": np.asarray(inputs["a1_proj_b"], f32),
        "wq2T": wT("a2_q_w"),
        "wkv2T": wT("a2_kv_w"),     # [Y, 2C]: cols 0:C = k, C:2C = v
        "wproj2T": wT("a2_proj_w"),
        "proj2b": np.asarray(inputs["a2_proj_b"], f32),
        "wfc1T": wT("m_fc1_w"),
        "fc1b": np.asarray(inputs["m_fc1_b"], f32),
        "wfc2T": wT("m_fc2_w"),
        "fc2b": np.asarray(inputs["m_fc2_b"], f32),
    }
    if not skip_gb:
        keymap = {"ln1": ("a1_ln1_g", "a1_ln1_b"), "aln2": ("a1_ln2_g", "a1_ln2_b"),
                  "a2ln": ("a2_ln_g", "a2_ln_b"), "mln1": ("m_ln1_g", "m_ln1_b"),
                  "mln2": ("m_ln2_g", "m_ln2_b")}
        for k, (gk, bk) in keymap.items():
            shared["g_" + k] = np.asarray(inputs[gk], f32)
            shared["b_" + k] = np.asarray(inputs[bk], f32)

    in_maps = []
    for core in range(G * CPG):
        g, c = core // CPG, core % CPG
        m = dict(shared)
        m["x_my"] = np.ascontiguousarray(x[g, c::CPG])
        m["x_full"] = np.ascontiguousarray(x[g])
        m["yT"] = np.ascontiguousarray(y[g].T)
        if mode == "causal":
            # boundary block: key kk (0..127) vs local row ii (0..31):
            # allowed iff kk <= c + 4*ii
            kk = np.arange(128)[:, None]
            ii = np.arange(32)[None, :]
            m["bmask"] = np.where(kk <= c + CPG * ii, 0.0, NEG).astype(f32)
        if mode == "dense":
            sub = mask[c::CPG, :]  # [R, N] rows of this core vs all keys
            m["maskT"] = np.ascontiguousarray(np.where(sub, NEG, 0.0).astype(f32).T)
        in_maps.append(m)
    return mode, skip_gb, in_maps


def _assemble(results, dtype):
    out = np.empty((B, N, C), np.float32)
    for core in range(G * CPG):
        g, c = core // CPG, core % CPG
        out[g, c::CPG] = results[core]["out_my"]
    return out.astype(dtype, copy=False)


def get_program(inputs):
    """Build (or fetch cached) program + per-core input maps for these inputs."""
    mode, skip_gb, in_maps = _host_prep(inputs)
    key = (mode, skip_gb)
    if key not in _CACHE:
        _CACHE[key] = _build(mode, skip_gb)
    return _CACHE[key], in_maps


def kernel(**inputs):
    from concourse import bass_utils

    nc, in_maps = get_program(inputs)
    res = bass_utils.run_bass_kernel_spmd(nc, in_maps, core_ids=list(range(8)))
    return _assemble(res.results, np.asarray(inputs["x"]).dtype)

